# revision 3
# baseline (speedup 1.0000x reference)
"""BERT-NER (12-layer BERT-base + token compaction + classifier) on 8 TRN2 cores.

Data-parallel over batch: 16 sequences -> 2 per core. Weights replicated,
host-converted to bf16 (halves HBM traffic; matmuls run bf16 with fp32 PSUM).
Activations are kept feature-major (xT: [768 partitions(6 tiles), 512 tokens])
so every x@W matmul uses the stored W[in,out] directly as lhsT.
PE-side overhead is minimized:
 - projection biases ride the ACT PSUM->SBUF copy (per-partition bias) or a
   fused DVE scalar_tensor_tensor (bias + residual in one op)
 - V bias is folded into the Wo bias on the host (exact: softmax rows sum to 1)
 - softmax denominators come free from the ctx matmul via a ones column
   appended to V (M=65); the per-query normalization uses a GpSimd
   partition_broadcast + one DVE multiply
 - LayerNorm stats still use ones-vector matmuls (cheap, already broadcast)
"""

import os
import sys

for _p in ("/opt/trn_rl_repo", "/root/.axon_site/_ro/trn_rl_repo"):
    if os.path.isdir(_p) and _p not in sys.path:
        sys.path.insert(0, _p)

import numpy as np
import ml_dtypes

import concourse.bass as bass
import concourse.mybir as mybir
import concourse.tile as tile
from concourse.tile import add_dep_helper
from concourse import bacc, bass_utils

F32 = mybir.dt.float32
F32R = mybir.dt.float32r
BF16 = mybir.dt.bfloat16
I32 = mybir.dt.int32
AF = mybir.ActivationFunctionType
ALU = mybir.AluOpType

B, S, H, L, A, V, NL = 16, 256, 768, 12, 12, 30522, 9
DH = H // A  # 64
FF = 4 * H  # 3072
NC = 8  # cores
BL = B // NC  # 2 sequences per core
T = BL * S  # 512 tokens per core
KT = H // 128  # 6 k-tiles of the hidden dim
TC = T // 128  # 4 token chunks
BIG = 1_000_000  # OOB dump index for compaction scatter
EPS = 1e-12
ISCALE = 1.0 / float(np.sqrt(DH))

P = 128


def build_nc(repeat=1, n_layers=L):
    nc = bacc.Bacc("TRN2", target_bir_lowering=False, debug=False)

    d_ids = nc.dram_tensor("input_word_ids", [BL, S], I32, kind="ExternalInput")
    d_mask = nc.dram_tensor("input_mask", [BL, S], I32, kind="ExternalInput")
    d_type = nc.dram_tensor("input_type_ids", [BL, S], I32, kind="ExternalInput")
    d_valid = nc.dram_tensor("valid_mask", [BL, S], I32, kind="ExternalInput")
    d_wemb = nc.dram_tensor("word_emb", [V, H], BF16, kind="ExternalInput")
    d_pemb = nc.dram_tensor("pos_emb", [S, H], BF16, kind="ExternalInput")
    d_temb = nc.dram_tensor("type_emb", [2, H], BF16, kind="ExternalInput")
    d_elng = nc.dram_tensor("emb_ln_g", [H], F32, kind="ExternalInput")
    d_elnb = nc.dram_tensor("emb_ln_b", [H], F32, kind="ExternalInput")
    d_Wq = nc.dram_tensor("Wq", [L, H, H], BF16, kind="ExternalInput")
    d_bq = nc.dram_tensor("bq", [L, H], F32, kind="ExternalInput")
    d_Wk = nc.dram_tensor("Wk", [L, H, H], BF16, kind="ExternalInput")
    d_bk = nc.dram_tensor("bk", [L, H], F32, kind="ExternalInput")
    d_Wv = nc.dram_tensor("Wv", [L, H, H], BF16, kind="ExternalInput")
    d_Wo = nc.dram_tensor("Wo", [L, H, H], BF16, kind="ExternalInput")
    d_bo = nc.dram_tensor("bo", [L, H], F32, kind="ExternalInput")
    d_alg = nc.dram_tensor("attn_ln_g", [L, H], F32, kind="ExternalInput")
    d_alb = nc.dram_tensor("attn_ln_b", [L, H], F32, kind="ExternalInput")
    d_W1 = nc.dram_tensor("W1", [L, H, FF], BF16, kind="ExternalInput")
    d_b1 = nc.dram_tensor("b1", [L, FF], F32, kind="ExternalInput")
    d_W2 = nc.dram_tensor("W2", [L, FF, H], BF16, kind="ExternalInput")
    d_b2 = nc.dram_tensor("b2", [L, H], F32, kind="ExternalInput")
    d_flg = nc.dram_tensor("ffn_ln_g", [L, H], F32, kind="ExternalInput")
    d_flb = nc.dram_tensor("ffn_ln_b", [L, H], F32, kind="ExternalInput")
    d_clsW = nc.dram_tensor("cls_W", [H, NL], BF16, kind="ExternalInput")
    d_clsb = nc.dram_tensor("cls_b", [NL], F32, kind="ExternalInput")
    d_out = nc.dram_tensor("out", [BL, S, NL], F32, kind="ExternalOutput")

    dr = dict(
        ids=d_ids, mask=d_mask, type=d_type, valid=d_valid, wemb=d_wemb,
        pemb=d_pemb, temb=d_temb, elng=d_elng, elnb=d_elnb,
        Wq=d_Wq, bq=d_bq, Wk=d_Wk, bk=d_bk, Wv=d_Wv, Wo=d_Wo, bo=d_bo,
        alg=d_alg, alb=d_alb, W1=d_W1, b1=d_b1, W2=d_W2, b2=d_b2,
        flg=d_flg, flb=d_flb, clsW=d_clsW, clsb=d_clsb, out=d_out,
    )

    with nc.allow_low_precision(reason="bf16 matmul pipeline"), tile.TileContext(
        nc
    ) as tc:
        with (
            tc.tile_pool(name="const", bufs=1) as cpool,
            tc.tile_pool(name="main", bufs=1) as mpool,
            tc.tile_pool(name="wts", bufs=6) as wpool,
            tc.tile_pool(name="w2p", bufs=10) as w2pool,
            tc.tile_pool(name="hrows", bufs=2) as rpool,
            tc.tile_pool(name="hbuf", bufs=4) as hpool,
            tc.tile_pool(name="ebuf", bufs=4) as epool,
            tc.tile_pool(name="small", bufs=2) as spool,
        ):
            pools = dict(c=cpool, m=mpool, w=wpool, w2=w2pool, r=rpool,
                         h=hpool, e=epool, s=spool)
            # ---- constants (device-generated) ----
            ident = cpool.tile([P, P], BF16, tag="ident")
            nc.gpsimd.memset(ident[:], 0.0)
            nc.gpsimd.affine_select(
                out=ident[:], in_=ident[:], compare_op=ALU.not_equal, fill=1.0,
                base=0, pattern=[[-1, P]], channel_multiplier=1,
            )
            ones_f32 = cpool.tile([P, 512], F32, tag="ones_f32")
            nc.gpsimd.memset(ones_f32[:], 1.0)
            ones128 = cpool.tile([P, P], BF16, tag="ones128")
            nc.vector.tensor_copy(out=ones128[:], in_=ones_f32[:, :P])
            ones_row = cpool.tile([1, 512], F32R, tag="ones_row")
            nc.vector.tensor_copy(out=ones_row[:], in_=ones_f32[:1, :])
            # lower-triangular-inclusive: ltri[p, ks, t] = 1 if (ks*128+p) <= t
            ltri_f = cpool.tile([P, 2, S], F32, tag="ltri_f")
            nc.gpsimd.memset(ltri_f[:], 1.0)
            nc.gpsimd.affine_select(
                out=ltri_f[:], in_=ltri_f[:], compare_op=ALU.is_ge, fill=0.0,
                base=0, pattern=[[-P, 2], [1, S]], channel_multiplier=-1,
            )
            consts = dict(ident=ident, ltri=ltri_f, ones_f32=ones_f32,
                          ones128=ones128, ones_row=ones_row)

            def body():
                emit_body(nc, tc, pools, consts, dr, n_layers)

            if repeat == 1:
                body()
            else:
                with tc.For_i(0, repeat, 1):
                    body()

    nc.compile()
    return nc


def _load_w_full(nc, wpool, d_slice):
    """Load a [H, 768] bf16 DRAM slice as SBUF [128, KT, 768] (k-tiles on
    partitions). Split across BOTH HWDGE engines (SP + Activation)."""
    w = wpool.tile([P, KT, H], BF16, tag="w_big", name="w_big")
    src = d_slice.rearrange("(kt p) c -> p kt c", p=P)
    nc.sync.dma_start(w[:, 0:3], src[:, 0:3])
    nc.scalar.dma_start(w[:, 3:6], src[:, 3:6])
    return w


def _bias_col(nc, spool, d_vec, tag):
    """Load [H] DRAM vector as [128, KT] (col m = slice m*128:(m+1)*128)."""
    t = spool.tile([P, KT], F32, tag=tag, name=tag)
    nc.sync.dma_start(t[:], d_vec.rearrange("(kt p) -> p kt", p=P))
    return t


def _bias_row(nc, rpool, d_vec, tag="brow", dtype=F32):
    """Load a DRAM vector [N<=768] as a single-partition row [1, N]."""
    n = d_vec.shape[0]
    t = rpool.tile([1, n], dtype, tag=tag, name=tag)
    nc.sync.dma_start(t[:], d_vec[None, :].bitcast(dtype))
    return t


def emit_ln(nc, tc, mpool, spool, y, g_col, b_col, out_tag, consts):
    """Feature-major layernorm on bf16 activations. Stat matmuls use an
    all-ones [128,128] lhsT so the per-token sums land already broadcast
    across 128 partitions; the stats chain then runs 128-lane on DVE/ACT."""
    ones128 = consts["ones128"]
    out = mpool.tile([P, KT, 512], BF16, tag=out_tag, name=out_tag)
    with tc.tile_pool(name="lnps", bufs=1, space="PSUM") as ppool:
        ps_s1 = ppool.tile([P, 512], F32, tag="ln_s1", space="PSUM")
        ps_s2 = ppool.tile([P, 512], F32, tag="ln_s2", space="PSUM")
        for kt in range(KT):
            sq = mpool.tile([P, 512], BF16, tag="ln_sq", bufs=2, name="sq")
            nc.scalar.activation(sq[:], y[:, kt], AF.Square)
            nc.tensor.matmul(ps_s1[:], ones128[:], y[:, kt],
                             start=(kt == 0), stop=(kt == KT - 1))
            nc.tensor.matmul(ps_s2[:], ones128[:], sq[:],
                             start=(kt == 0), stop=(kt == KT - 1))
        mean = spool.tile([P, 512], F32, tag="ln_mean", bufs=1, name="ln_mean")
        nc.vector.tensor_scalar_mul(mean[:], ps_s1[:], 1.0 / H)
        m2 = spool.tile([P, 512], F32, tag="ln_m2", bufs=1, name="ln_m2")
        nc.vector.tensor_tensor(out=m2[:], in0=mean[:], in1=mean[:], op=ALU.mult)
        var = spool.tile([P, 512], F32, tag="ln_var", bufs=1, name="ln_var")
        nc.vector.tensor_scalar(out=var[:], in0=ps_s2[:], scalar1=1.0 / H,
                                scalar2=EPS, op0=ALU.mult, op1=ALU.add)
        nc.vector.tensor_tensor(out=var[:], in0=var[:], in1=m2[:], op=ALU.subtract)
        std = spool.tile([P, 512], F32, tag="ln_std", bufs=1, name="ln_std")
        nc.scalar.activation(std[:], var[:], AF.Sqrt)
        rstd = spool.tile([P, 512], F32, tag="ln_rstd", bufs=1, name="ln_rstd")
        nc.vector.reciprocal(rstd[:], std[:])
        for kt in range(KT):
            tmp = mpool.tile([P, 512], F32, tag="ln_tmp", bufs=2, name="tmp")
            nc.vector.tensor_tensor(out=tmp[:], in0=y[:, kt], in1=mean[:],
                                    op=ALU.subtract)
            nc.vector.tensor_tensor(out=tmp[:], in0=tmp[:], in1=rstd[:],
                                    op=ALU.mult)
            nc.scalar.activation(out[:, kt], tmp[:], AF.Identity,
                                 scale=g_col[:, kt : kt + 1],
                                 bias=b_col[:, kt : kt + 1])
    return out


def emit_body(nc, tc, pools, consts, dr, n_layers):
    cpool, mpool, wpool, w2pool = (
        pools["c"], pools["m"], pools["w"], pools["w2"])
    rpool, hpool, epool, spool = (
        pools["r"], pools["h"], pools["e"], pools["s"])
    ident, ltri = consts["ident"], consts["ltri"]
    ones_f32 = consts["ones_f32"]
    ones_row = consts["ones_row"]
    ones128 = consts["ones128"]

    ids_flat = dr["ids"].rearrange("b s -> (b s)")
    type_flat = dr["type"].rearrange("b s -> (b s)")
    mask_flat = dr["mask"].rearrange("b s -> (b s)")
    valid_flat = dr["valid"].rearrange("b s -> (b s)")

    # amask[:, c]: 0 where mask==1 else -10000 ; valid_f: valid mask as f32
    amask = cpool.tile([P, TC], F32, tag="amask", name="amask")
    valid_f = cpool.tile([P, TC], F32, tag="valid_f", name="valid_f")

    # V tiles carry an appended ones column so the ctx matmul also emits the
    # softmax denominator (row 64 of the 65-row output).
    vsb = mpool.tile([P, TC, A, DH + 1], BF16, tag="vsb", name="vsb")
    nc.gpsimd.memset(vsb[:], 1.0)

    # ============ embeddings (token-major), transpose, LN ============
    xtok = mpool.tile([P, TC, H], BF16, tag="bigA", name="xtok")
    for c in range(TC):
        idt = spool.tile([P, 1], I32, tag="idt", name="idt")
        nc.sync.dma_start(idt[:], ids_flat[c * P : (c + 1) * P, None])
        nc.gpsimd.indirect_dma_start(
            out=xtok[:, c], out_offset=None, in_=dr["wemb"][:, :],
            in_offset=bass.IndirectOffsetOnAxis(ap=idt[:, :1], axis=0),
        )
        tyt = spool.tile([P, 1], I32, tag="tyt", name="tyt")
        nc.sync.dma_start(tyt[:], type_flat[c * P : (c + 1) * P, None])
        temb = hpool.tile([P, H], BF16, tag="temb", bufs=2, name="temb")
        nc.gpsimd.indirect_dma_start(
            out=temb[:], out_offset=None, in_=dr["temb"][:, :],
            in_offset=bass.IndirectOffsetOnAxis(ap=tyt[:, :1], axis=0),
        )
        pemb = hpool.tile([P, H], BF16, tag="pemb", bufs=2, name="pemb")
        cc = c % (S // P)
        nc.sync.dma_start(pemb[:], dr["pemb"][cc * P : (cc + 1) * P, :])
        nc.vector.tensor_tensor(out=xtok[:, c], in0=xtok[:, c], in1=temb[:],
                                op=ALU.add)
        nc.vector.tensor_tensor(out=xtok[:, c], in0=xtok[:, c], in1=pemb[:],
                                op=ALU.add)

        mi = spool.tile([P, 1], I32, tag="mi", name="mi")
        nc.sync.dma_start(mi[:], mask_flat[c * P : (c + 1) * P, None])
        mf = spool.tile([P, 1], F32, tag="mf", name="mf")
        nc.vector.tensor_copy(out=mf[:], in_=mi[:])
        nc.scalar.activation(amask[:, c : c + 1], mf[:], AF.Copy,
                             scale=10000.0, bias=-10000.0)
        vi = spool.tile([P, 1], I32, tag="vi", name="vi")
        nc.sync.dma_start(vi[:], valid_flat[c * P : (c + 1) * P, None])
        nc.vector.tensor_copy(out=valid_f[:, c : c + 1], in_=vi[:])

    # transpose to feature-major
    xe = mpool.tile([P, KT, 512], BF16, tag="bigB", name="xe")
    with tc.tile_pool(name="embtr", bufs=2, space="PSUM") as ppool:
        for kt in range(KT):
            for c in range(TC):
                ps_t = ppool.tile([P, P], BF16, tag="tr", space="PSUM")
                nc.tensor.transpose(
                    out=ps_t[:], in_=xtok[:, c, kt * P : (kt + 1) * P],
                    identity=ident[:])
                nc.vector.tensor_copy(out=xe[:, kt, c * P : (c + 1) * P],
                                      in_=ps_t[:])
    eg = _bias_col(nc, spool, dr["elng"][:], "eg")
    eb = _bias_col(nc, spool, dr["elnb"][:], "eb")
    x = emit_ln(nc, tc, mpool, spool, xe, eg, eb, "x_cur", consts)

    # ============ transformer layers ============
    for l in range(n_layers):
        # ---- Q, K projections (feature-major out, bias on ACT copy) ----
        qT = mpool.tile([P, KT, 512], BF16, tag="bigA", name="qT")
        kT = mpool.tile([P, KT, 512], BF16, tag="bigB", name="kT")
        with tc.tile_pool(name=f"qkv{l}", bufs=2, space="PSUM") as ppool:
            for name, d_W, d_b, dst in (("q", dr["Wq"], dr["bq"], qT),
                                        ("k", dr["Wk"], dr["bk"], kT)):
                bc = _bias_col(nc, spool, d_b[l], f"b_{name}")
                w = _load_w_full(nc, wpool, d_W[l])
                for m in range(KT):
                    ps = ppool.tile([P, 512], F32, tag="qk_ps", bufs=4,
                                    space="PSUM")
                    for kt in range(KT):
                        nc.tensor.matmul(ps[:],
                                         w[:, kt, m * P : (m + 1) * P],
                                         x[:, kt], start=(kt == 0),
                                         stop=(kt == KT - 1))
                    nc.scalar.activation(dst[:, m], ps[:], AF.Identity,
                                         bias=bc[:, m : m + 1])
            # ---- V (token-major; bias folded into bo on the host) ----
            wv = _load_w_full(nc, wpool, dr["Wv"][l])
            for hh in range(2):
                for c in range(TC):
                    ps = ppool.tile([P, 384], F32, tag="v_ps", bufs=4,
                                    space="PSUM")
                    for kt in range(KT):
                        nc.tensor.matmul(
                            ps[:], x[:, kt, c * P : (c + 1) * P],
                            wv[:, kt, hh * 384 : (hh + 1) * 384],
                            start=(kt == 0), stop=(kt == KT - 1))
                    nc.vector.tensor_copy(
                        out=vsb[:, c, hh * 6 : (hh + 1) * 6, :DH],
                        in_=ps[:].rearrange("p (h d) -> p h d", d=DH))

        # ---- attention per (seq, head) ----
        ctxT = mpool.tile([P, KT, 512], BF16, tag="bigC", name="ctxT")
        with tc.tile_pool(name=f"att{l}", bufs=2, space="PSUM") as ppool:
            for b in range(BL):
                for h in range(A):
                    ft, fo = h // 2, (h % 2) * DH
                    ps_s = ppool.tile([P, 2, S], F32, tag="s_ps", bufs=3,
                                      space="PSUM")
                    esb = []
                    for kc in range(2):
                        nc.tensor.matmul(
                            ps_s[:, kc],
                            kT[fo : fo + DH, ft,
                               b * S + kc * P : b * S + (kc + 1) * P],
                            qT[fo : fo + DH, ft, b * S : (b + 1) * S],
                            start=True, stop=True)
                        e = epool.tile([P, S], BF16, tag="e_sb", bufs=6,
                                       name="e_sb")
                        nc.scalar.activation(
                            e[:], ps_s[:, kc], AF.Exp, scale=ISCALE,
                            bias=amask[:, b * 2 + kc : b * 2 + kc + 1])
                        esb.append(e)
                    # ctx + denominator in one accumulation group (M=65)
                    ps_c = ppool.tile([DH + 1, S], F32, tag="c_ps", bufs=3,
                                      space="PSUM")
                    for kc in range(2):
                        nc.tensor.matmul(ps_c[:], vsb[:, b * 2 + kc, h, :],
                                         esb[kc][:], start=(kc == 0),
                                         stop=(kc == 1))
                    rsum = spool.tile([1, S], F32R, tag="rsum", bufs=4,
                                      name="rsum")
                    nc.vector.reciprocal(rsum[:], ps_c[DH : DH + 1, :])
                    # broadcast recip across DH partitions via a rank-1 matmul
                    # (f32r: full PE rate at N>=256)
                    ps_b = ppool.tile([DH, S], F32, tag="b_ps", bufs=2,
                                      space="PSUM")
                    nc.tensor.matmul(ps_b[:], ones_row[:1, :DH], rsum[:],
                                     start=True, stop=True)
                    rb = epool.tile([DH, S], BF16, tag="rb", bufs=4, name="rb")
                    nc.vector.tensor_copy(out=rb[:], in_=ps_b[:])
                    nc.vector.tensor_tensor(
                        out=ctxT[fo : fo + DH, ft, b * S : (b + 1) * S],
                        in0=ps_c[:DH, :], in1=rb[:], op=ALU.mult)

        # ---- output projection + bias + residual + LN1 ----
        y1 = mpool.tile([P, KT, 512], BF16, tag="bigA", name="y1")
        with tc.tile_pool(name=f"wo{l}", bufs=2, space="PSUM") as ppool:
            boc = _bias_col(nc, spool, dr["bo"][l], "b_o")
            wo = _load_w_full(nc, wpool, dr["Wo"][l])
            for m in range(KT):
                ps = ppool.tile([P, 512], F32, tag="o_ps", bufs=3,
                                space="PSUM")
                for kt in range(KT):
                    nc.tensor.matmul(ps[:], wo[:, kt, m * P : (m + 1) * P],
                                     ctxT[:, kt], start=(kt == 0),
                                     stop=(kt == KT - 1))
                nc.vector.scalar_tensor_tensor(
                    out=y1[:, m], in0=ps[:], scalar=boc[:, m : m + 1],
                    in1=x[:, m], op0=ALU.add, op1=ALU.add)
            ag = _bias_col(nc, spool, dr["alg"][l], "ag")
            ab = _bias_col(nc, spool, dr["alb"][l], "ab")
            x2 = emit_ln(nc, tc, mpool, spool, y1, ag, ab, "bigB", consts)

        # ---- FFN (y2 accumulated in 6 persistent PSUM banks) ----
        y3 = mpool.tile([P, KT, 512], BF16, tag="bigC", name="y3")
        with (
            tc.tile_pool(name=f"ffa{l}", bufs=1, space="PSUM") as papool,
            tc.tile_pool(name=f"ffh{l}", bufs=2, space="PSUM") as ppool,
        ):
            b2c = _bias_col(nc, spool, dr["b2"][l], "b_2")
            ps_y = [papool.tile([P, 512], F32, tag=f"acc{m}",
                                name=f"ps_y{l}_{m}", space="PSUM")
                    for m in range(KT)]
            for q4 in range(4):
                w1 = _load_w_full(nc, wpool, dr["W1"][l][:, q4 * H : (q4 + 1) * H])
                b1c = _bias_col(nc, spool, dr["b1"][l][q4 * H : (q4 + 1) * H],
                                "b_1")
                for cc in range(KT):
                    c = q4 * KT + cc
                    ps_h = ppool.tile([P, 512], F32, tag="h_ps", space="PSUM")
                    for kt in range(KT):
                        nc.tensor.matmul(ps_h[:],
                                         w1[:, kt, cc * P : (cc + 1) * P],
                                         x2[:, kt], start=(kt == 0),
                                         stop=(kt == KT - 1))
                    hsb = hpool.tile([P, 512], BF16, tag="h_sb", name="hsb")
                    nc.scalar.activation(hsb[:], ps_h[:], AF.Gelu_apprx_tanh,
                                         bias=b1c[:, cc : cc + 1])
                    w2 = w2pool.tile([P, H], BF16, tag="w2c", name="w2")
                    w2eng = nc.sync if c % 2 == 0 else nc.scalar
                    w2eng.dma_start(w2[:], dr["W2"][l][c * P : (c + 1) * P, :])
                    for m in range(KT):
                        nc.tensor.matmul(ps_y[m][:], w2[:, m * P : (m + 1) * P],
                                         hsb[:], start=(c == 0),
                                         stop=(c == FF // P - 1))
            for m in range(KT):
                nc.vector.scalar_tensor_tensor(
                    out=y3[:, m], in0=ps_y[m][:], scalar=b2c[:, m : m + 1],
                    in1=x2[:, m], op0=ALU.add, op1=ALU.add)
        fg = _bias_col(nc, spool, dr["flg"][l], "fg")
        fb = _bias_col(nc, spool, dr["flb"][l], "fb")
        x = emit_ln(nc, tc, mpool, spool, y3, fg, fb, "x_cur", consts)

    # ============ classifier head + softmax + compaction ============
    with tc.tile_pool(name="head", bufs=2, space="PSUM") as ppool:
        clsw = spool.tile([P, KT, NL], BF16, tag="clsw", name="clsw")
        nc.sync.dma_start(
            clsw[:], dr["clsW"].rearrange("(kt p) c -> p kt c", p=P))
        clsb = _bias_row(nc, rpool, dr["clsb"][:], tag="clsb", dtype=F32)

        # uniform pad row: softmax(cls_b), broadcast to 128 partitions
        nmx = spool.tile([1, 1], F32, tag="nmx", name="nmx")
        nc.vector.reduce_max(out=nmx[:], in_=clsb[:], negate=True,
                             axis=mybir.AxisListType.X)
        usum = spool.tile([1, 1], F32, tag="usum", name="usum")
        uex = spool.tile([1, NL], F32, tag="uex", name="uex")
        nc.scalar.activation(uex[:], clsb[:], AF.Exp, bias=nmx[:],
                             accum_out=usum[:])
        urs = spool.tile([1, 1], F32, tag="urs", name="urs")
        nc.vector.reciprocal(urs[:], usum[:])
        uni = spool.tile([1, NL], F32, tag="uni", name="uni")
        nc.vector.tensor_scalar_mul(uni[:], uex[:], urs[:])
        ps_u = ppool.tile([P, NL], F32, tag="u_ps", space="PSUM")
        nc.tensor.matmul(ps_u[:], ones_f32[:1, :P], uni[:], start=True, stop=True)
        uni128 = spool.tile([P, NL], F32, tag="uni128", name="uni128")
        nc.vector.tensor_copy(out=uni128[:], in_=ps_u[:])
        out_flat = dr["out"].rearrange("b s c -> (b s) c")
        prefills = []
        for c in range(TC):
            dma = nc.sync.dma_start(out_flat[c * P : (c + 1) * P, :], uni128[:])
            prefills.append(dma.ins)

        clsbf = spool.tile([1, NL], BF16, tag="clsbf", name="clsbf")
        nc.vector.tensor_copy(out=clsbf[:], in_=clsb[:])
        for c in range(TC):
            b = c // (S // P)
            ps_lg = ppool.tile([P, NL], F32, tag="lg_ps", space="PSUM")
            for kt in range(KT):
                nc.tensor.matmul(ps_lg[:], x[:, kt, c * P : (c + 1) * P],
                                 clsw[:, kt], start=(kt == 0), stop=False)
            nc.tensor.matmul(ps_lg[:], ones128[:1, :], clsbf[:],
                             start=False, stop=True)
            negmax = spool.tile([P, 1], F32, tag="negmax", name="negmax")
            nc.vector.reduce_max(out=negmax[:], in_=ps_lg[:], negate=True,
                                 axis=mybir.AxisListType.X)
            probs = spool.tile([P, NL], F32, tag="probs", name="probs")
            sm = spool.tile([P, 1], F32, tag="sm", name="sm")
            nc.scalar.activation(probs[:], ps_lg[:], AF.Exp, bias=negmax[:],
                                 accum_out=sm[:])
            rs = spool.tile([P, 1], F32, tag="rs", name="rs")
            nc.vector.reciprocal(rs[:], sm[:])
            nc.vector.tensor_scalar_mul(probs[:], probs[:], rs[:])

            # cumsum of valid over the sequence, sliced to this chunk
            cc = c % (S // P)
            ps_cs = ppool.tile([P, 1], F32, tag="cs_ps", space="PSUM")
            for ks in range(2):
                nc.tensor.matmul(ps_cs[:], ltri[:, ks, cc * P : (cc + 1) * P],
                                 valid_f[:, b * 2 + ks : b * 2 + ks + 1],
                                 start=(ks == 0), stop=(ks == 1))
            # dest = valid ? b*S + csum - 1 : BIG
            dest_f = spool.tile([P, 1], F32, tag="dest_f", name="dest_f")
            nc.vector.tensor_scalar_add(dest_f[:], ps_cs[:], float(b * S - 1 - BIG))
            nc.vector.tensor_tensor(out=dest_f[:], in0=dest_f[:],
                                    in1=valid_f[:, c : c + 1], op=ALU.mult)
            nc.vector.tensor_scalar_add(dest_f[:], dest_f[:], float(BIG))
            dest_i = spool.tile([P, 1], I32, tag="dest_i", name="dest_i")
            nc.vector.tensor_copy(out=dest_i[:], in_=dest_f[:])

            scat = nc.gpsimd.indirect_dma_start(
                out=out_flat[:, :],
                out_offset=bass.IndirectOffsetOnAxis(ap=dest_i[:, :1], axis=0),
                in_=probs[:],
                in_offset=None,
                bounds_check=T - 1, oob_is_err=False,
            )
            for pf in prefills:
                add_dep_helper(scat.ins, pf,
                               reason="scatter after uniform prefill")


_NC_CACHE = {}


def _get_nc(repeat=1, n_layers=L):
    key = (repeat, n_layers)
    if key not in _NC_CACHE:
        _NC_CACHE[key] = build_nc(repeat=repeat, n_layers=n_layers)
    return _NC_CACHE[key]


_BF16_KEYS = ("word_emb", "pos_emb", "type_emb", "Wq", "Wk", "Wv", "Wo",
              "W1", "W2")


def make_in_maps(inputs):
    per_seq = {}
    for name in ("input_word_ids", "input_mask", "input_type_ids", "valid_mask"):
        per_seq[name] = np.ascontiguousarray(np.asarray(inputs[name]))
    shared = {}
    f32 = {k: np.asarray(v, dtype=np.float32) for k, v in inputs.items()
           if k not in per_seq}
    # exact fold: softmax rows sum to 1, so ctx@Wo picks up bv@Wo per token
    bo_folded = f32["bo"] + np.einsum("lh,lhm->lm", f32["bv"], f32["Wo"])
    f32["bo"] = bo_folded.astype(np.float32)
    del f32["bv"]
    for k, v in f32.items():
        if k in _BF16_KEYS or k == "cls_W":
            shared[k] = np.ascontiguousarray(v.astype(ml_dtypes.bfloat16))
        else:
            shared[k] = np.ascontiguousarray(v)
    in_maps = []
    for c in range(NC):
        m = dict(shared)
        for name, arr in per_seq.items():
            m[name] = np.ascontiguousarray(arr[c * BL : (c + 1) * BL])
        in_maps.append(m)
    return in_maps


def kernel(**inputs):
    nc = _get_nc()
    in_maps = make_in_maps(inputs)
    res = bass_utils.run_bass_kernel_spmd(nc, in_maps, list(range(NC)))
    out = np.concatenate([res.results[c]["out"] for c in range(NC)], axis=0)
    return out.astype(np.float32)
